# revision 1
# baseline (speedup 1.0000x reference)
"""nn_CrossAttention kernel — data-parallel over batch B=8 across 8 NeuronCores.

Takes FULL unsharded inputs, returns FULL output [8, 64, 64, 512] float32.
Strategy (per sharding_hint): shard batch dim across the 8 cores; each core
runs the full linear -> dual-LN -> dual-softmax cross-attention -> 1x1
reprojection -> LayerNorm pipeline for its batch element; gather at the end.
"""

import numpy as np

B, H, W = 8, 64, 64
D = 256
HEADS = 8
DK = D // HEADS
N = H * W
EPS = 1e-5


def _forward_jax(jnp, jax, x1, x2, linear_w, linear_b, ln1_g, ln1_b,
                 reproj_w, reproj_b, ln_attn_g, ln_attn_b):
    """Per-shard forward. x1: [b, H, W, 2D], x2: [b, H, W, D]."""
    b = x1.shape[0]

    def _ln(x, g, bb):
        m = jnp.mean(x, axis=-1, keepdims=True)
        v = jnp.var(x, axis=-1, keepdims=True)
        return (x - m) * jax.lax.rsqrt(v + EPS) * g + bb

    n1 = _ln(x1 @ linear_w + linear_b, ln1_g, ln1_b)
    n2 = _ln(x2, ln1_g, ln1_b)
    v = n1.reshape(b, N, D).transpose(0, 2, 1).reshape(b, HEADS, DK, N)
    kq = n2.reshape(b, N, D).transpose(0, 2, 1).reshape(b, HEADS, DK, N)
    k = jax.nn.softmax(kq, axis=-1)
    q = jax.nn.softmax(kq, axis=2)
    ctx = jnp.einsum('bhdm,bhem->bhde', q, k)
    att = jnp.einsum('bhde,bhen->bhdn', ctx, v)
    agg = att.reshape(b, D, H, W)
    rep = jnp.einsum('od,bdhw->bohw', reproj_w, agg) \
        + reproj_b[None, :, None, None]
    rep = rep.transpose(0, 2, 3, 1)
    return x1 + _ln(rep, ln_attn_g, ln_attn_b)


def _kernel_trn(inputs):
    """Data-parallel pmap over 8 NeuronCores: batch shard of 1 per core."""
    import jax
    import jax.numpy as jnp

    devs = jax.devices()[:8]
    assert len(devs) == 8

    def shard_fn(x1, x2, lw, lb, g1, b1, rw, rb, ga, ba):
        return _forward_jax(jnp, jax, x1, x2, lw, lb, g1, b1, rw, rb, ga, ba)

    pm = jax.pmap(shard_fn, devices=devs,
                  in_axes=(0, 0, None, None, None, None, None, None, None, None))
    # [8, 1, H, W, C] shards: one batch element per core
    x1s = inputs['x1'].reshape(8, 1, H, W, 2 * D)
    x2s = inputs['x2'].reshape(8, 1, H, W, D)
    out = pm(x1s, x2s, inputs['linear_w'], inputs['linear_b'],
             inputs['ln1_g'], inputs['ln1_b'], inputs['reproj_w'],
             inputs['reproj_b'], inputs['ln_attn_g'], inputs['ln_attn_b'])
    return np.asarray(out).reshape(B, H, W, 2 * D).astype(np.float32)


def _kernel_numpy(inputs):
    """CPU fallback, exact reference math in float32."""
    x1 = np.asarray(inputs['x1'], np.float32)
    x2 = np.asarray(inputs['x2'], np.float32)
    lw = np.asarray(inputs['linear_w'], np.float32)
    lb = np.asarray(inputs['linear_b'], np.float32)
    g1 = np.asarray(inputs['ln1_g'], np.float32)
    b1 = np.asarray(inputs['ln1_b'], np.float32)
    rw = np.asarray(inputs['reproj_w'], np.float32)
    rb = np.asarray(inputs['reproj_b'], np.float32)
    ga = np.asarray(inputs['ln_attn_g'], np.float32)
    ba = np.asarray(inputs['ln_attn_b'], np.float32)

    def _ln(x, g, bb):
        m = x.mean(-1, keepdims=True)
        v = x.var(-1, keepdims=True)
        return (x - m) / np.sqrt(v + EPS) * g + bb

    def _softmax(x, axis):
        x = x - x.max(axis=axis, keepdims=True)
        e = np.exp(x)
        return e / e.sum(axis=axis, keepdims=True)

    n1 = _ln(x1 @ lw + lb, g1, b1)
    n2 = _ln(x2, g1, b1)
    v = n1.reshape(B, N, D).transpose(0, 2, 1).reshape(B, HEADS, DK, N)
    kq = n2.reshape(B, N, D).transpose(0, 2, 1).reshape(B, HEADS, DK, N)
    k = _softmax(kq, -1)
    q = _softmax(kq, 2)
    ctx = np.einsum('bhdm,bhem->bhde', q, k)
    att = np.einsum('bhde,bhen->bhdn', ctx, v)
    agg = att.reshape(B, D, H, W)
    rep = np.einsum('od,bdhw->bohw', rw, agg) + rb[None, :, None, None]
    rep = rep.transpose(0, 2, 3, 1)
    return (x1 + _ln(rep, ga, ba)).astype(np.float32)


def kernel(**inputs):
    try:
        return _kernel_trn(inputs)
    except Exception:
        return _kernel_numpy(inputs)


# revision 2
# speedup vs baseline: 1.3536x; 1.3536x over previous
"""nn_CrossAttention kernel — data-parallel over batch B=8 across 8 NeuronCores.

Takes FULL unsharded inputs, returns FULL output [8, 64, 64, 512] float32.
Strategy (per sharding_hint): shard batch dim across the 8 cores; each core
runs the full linear -> dual-LN -> dual-softmax cross-attention -> 1x1
reprojection -> LayerNorm pipeline for its batch element; gather at the end.
"""

import numpy as np

B, H, W = 8, 64, 64
D = 256
HEADS = 8
DK = D // HEADS
N = H * W
EPS = 1e-5


def _forward_jax(jnp, jax, x1, x2, linear_w, linear_b, ln1_g, ln1_b,
                 reproj_w, reproj_b, ln_attn_g, ln_attn_b):
    """Per-shard forward. x1: [b, H, W, 2D], x2: [b, H, W, D]."""
    b = x1.shape[0]

    def _ln(x, g, bb):
        m = jnp.mean(x, axis=-1, keepdims=True)
        v = jnp.var(x, axis=-1, keepdims=True)
        return (x - m) * jax.lax.rsqrt(v + EPS) * g + bb

    n1 = _ln(x1 @ linear_w + linear_b, ln1_g, ln1_b)
    n2 = _ln(x2, ln1_g, ln1_b)
    v = n1.reshape(b, N, D).transpose(0, 2, 1).reshape(b, HEADS, DK, N)
    kq = n2.reshape(b, N, D).transpose(0, 2, 1).reshape(b, HEADS, DK, N)
    k = jax.nn.softmax(kq, axis=-1)
    q = jax.nn.softmax(kq, axis=2)
    ctx = jnp.einsum('bhdm,bhem->bhde', q, k)
    att = jnp.einsum('bhde,bhen->bhdn', ctx, v)
    agg = att.reshape(b, D, H, W)
    rep = jnp.einsum('od,bdhw->bohw', reproj_w, agg) \
        + reproj_b[None, :, None, None]
    rep = rep.transpose(0, 2, 3, 1)
    return x1 + _ln(rep, ln_attn_g, ln_attn_b)


_PMAP_CACHE = {}


def _get_pmap():
    if 'pm' in _PMAP_CACHE:
        return _PMAP_CACHE['pm']
    import jax
    import jax.numpy as jnp

    devs = jax.devices()[:8]
    assert len(devs) == 8

    def shard_fn(x1, x2, lw, lb, g1, b1, rw, rb, ga, ba):
        return _forward_jax(jnp, jax, x1, x2, lw, lb, g1, b1, rw, rb, ga, ba)

    pm = jax.pmap(shard_fn, devices=devs,
                  in_axes=(0, 0, None, None, None, None, None, None, None, None))
    _PMAP_CACHE['pm'] = pm
    return pm


def _kernel_trn(inputs):
    """Data-parallel pmap over 8 NeuronCores: batch shard of 1 per core."""
    pm = _get_pmap()
    # [8, 1, H, W, C] shards: one batch element per core
    x1s = inputs['x1'].reshape(8, 1, H, W, 2 * D)
    x2s = inputs['x2'].reshape(8, 1, H, W, D)
    out = pm(x1s, x2s, inputs['linear_w'], inputs['linear_b'],
             inputs['ln1_g'], inputs['ln1_b'], inputs['reproj_w'],
             inputs['reproj_b'], inputs['ln_attn_g'], inputs['ln_attn_b'])
    return np.asarray(out).reshape(B, H, W, 2 * D).astype(np.float32)


def _kernel_numpy(inputs):
    """CPU fallback, exact reference math in float32."""
    x1 = np.asarray(inputs['x1'], np.float32)
    x2 = np.asarray(inputs['x2'], np.float32)
    lw = np.asarray(inputs['linear_w'], np.float32)
    lb = np.asarray(inputs['linear_b'], np.float32)
    g1 = np.asarray(inputs['ln1_g'], np.float32)
    b1 = np.asarray(inputs['ln1_b'], np.float32)
    rw = np.asarray(inputs['reproj_w'], np.float32)
    rb = np.asarray(inputs['reproj_b'], np.float32)
    ga = np.asarray(inputs['ln_attn_g'], np.float32)
    ba = np.asarray(inputs['ln_attn_b'], np.float32)

    def _ln(x, g, bb):
        m = x.mean(-1, keepdims=True)
        v = x.var(-1, keepdims=True)
        return (x - m) / np.sqrt(v + EPS) * g + bb

    def _softmax(x, axis):
        x = x - x.max(axis=axis, keepdims=True)
        e = np.exp(x)
        return e / e.sum(axis=axis, keepdims=True)

    n1 = _ln(x1 @ lw + lb, g1, b1)
    n2 = _ln(x2, g1, b1)
    v = n1.reshape(B, N, D).transpose(0, 2, 1).reshape(B, HEADS, DK, N)
    kq = n2.reshape(B, N, D).transpose(0, 2, 1).reshape(B, HEADS, DK, N)
    k = _softmax(kq, -1)
    q = _softmax(kq, 2)
    ctx = np.einsum('bhdm,bhem->bhde', q, k)
    att = np.einsum('bhde,bhen->bhdn', ctx, v)
    agg = att.reshape(B, D, H, W)
    rep = np.einsum('od,bdhw->bohw', rw, agg) + rb[None, :, None, None]
    rep = rep.transpose(0, 2, 3, 1)
    return (x1 + _ln(rep, ga, ba)).astype(np.float32)


def kernel(**inputs):
    try:
        return _kernel_trn(inputs)
    except Exception:
        return _kernel_numpy(inputs)
